# revision 13
# baseline (speedup 1.0000x reference)
"""Batched conv layer (im2col gather + einsum) as a Bass/Tile TRN2 kernel.

Problem: x (8,16,32,32,64) f32, kernel (8,3,3,64,128) f32
         out[b,i,oh,ow,f] = sum_{kh,kw,c} xpad[b,i,oh+kh-1,ow+kw-1,c] * kernel[b,kh,kw,c,f]
         out (8,16,32,32,128) f32

Sharding: batch dim b across 8 cores (pure data parallel, no collectives).

Per-core device layout (host prepares these):
  xp : (8 pairs, 128, 34*34) f32   partition dim packs 2 images x 64 channels;
                                   free dim is the zero-padded 34x34 image plane
  kd : (128, 9*128) f32            partition dim packs 2 copies of the 64 channels
                                   (one per image in a pair); free dim is
                                   9 taps x 128 output filters
  out: (16, 128, 1024) f32         [image, filter, position]; host transposes back

The conv is computed as 9 shifted matmuls accumulated in PSUM:
  out[f, pos] += ktap[c, f].T @ xwin[c, pos]   for each tap (kh, kw)
Images are processed in pairs occupying PE row-groups 0-63 / 64-127 so two
K=64 matmuls can run concurrently in the 128x128 array.
"""

import os

import numpy as np

import concourse.bass as bass
import concourse.mybir as mybir
from concourse import bacc
from concourse.bass_utils import run_bass_kernel_spmd
from concourse.tile import TileContext

# Static problem config (hardcoded per the harness contract)
B, I, H, W, C, F = 8, 16, 32, 32, 64, 128
KD = 3
HP = H + 2  # padded
WP = W + 2
NPOS = H * W          # 1024 output positions per image
NTILE = 512           # positions per PSUM tile (one bank)
NHALF = NPOS // NTILE  # 2
ROWS_PER_TILE = NTILE // W  # 16 output rows per tile
N_CORES = 8

# matmul input dtype: "f16" (default: ~3e-4 rel err, fastest), "f32r"
# (~1.5e-4), "f32" (exact, 4x slower PE), "bf16"
MM_DTYPE = os.environ.get("CONV_MM_DTYPE", "f16")
# weight (stationary operand) dtype: "" = same as MM_DTYPE
W_DTYPE = os.environ.get("CONV_W_DTYPE", "")
# store outputs as f16 (host casts back to f32): halves the 8.4 MB of
# output HBM traffic and the PSUM->SBUF copy time.  Quantization adds
# ~3e-4 rel err on top of the f16-matmul ~3e-4 — far under the 2e-2 gate.
OUT_F16 = os.environ.get("CONV_OUT_F16", "1") == "1"
# PE warm-up matmuls (see below); "1" = on.
WARMUP = os.environ.get("CONV_WARMUP", "1") == "1"
N_WARM = max(4, int(os.environ.get("CONV_N_WARM", "7")))

_CACHED_NC = None
LAST_RESULTS = None


def _build_nc():
    nc = bacc.Bacc(trn_type="TRN2")

    mm_dt = {
        "f32": mybir.dt.float32,
        "f32r": mybir.dt.float32r,
        "bf16": mybir.dt.bfloat16,
        "f16": mybir.dt.float16,
    }[MM_DTYPE]
    # For f32r, type the DRAM inputs as float32r end-to-end (same 4-byte fp32
    # layout; the PE just reads fewer mantissa bits) so the BIR verifier sees a
    # consistent fp32r producer chain.  For f16 the host pre-casts the inputs.
    if MM_DTYPE in ("f32r", "f16"):
        in_dt = mm_dt
    else:
        in_dt = mybir.dt.float32

    k_dt = mybir.dt.float16 if W_DTYPE == "f16" else in_dt

    out_dt = mybir.dt.float16 if OUT_F16 else mybir.dt.float32

    xp = nc.declare_dram_parameter("xp", [I // 2, 128, HP * WP], in_dt, isOutput=False)
    kd = nc.declare_dram_parameter("kd", [128, KD * KD * F], k_dt, isOutput=False)
    out = nc.declare_dram_parameter("out", [I, F, NPOS], out_dt, isOutput=True)

    with TileContext(nc) as tc:
        with (
            tc.tile_pool(name="kpool", bufs=1) as kpool,
            tc.tile_pool(name="xpool", bufs=8) as xpool,
            tc.tile_pool(name="opool", bufs=32) as opool,
            tc.tile_pool(name="psum", bufs=8, space="PSUM") as psum_pool,
        ):
            # PE warm-up: the HAM clock gate runs the PE at 1.2 GHz until it
            # has seen ~3.4us of sustained activity; the first real matmul
            # can't start before ~9.8us (framework preamble ~6.8us + first
            # input DMA ~3us).  Burn zero-valued matmuls on a memset tile
            # during that dead window so the HAM flips to 2.4 GHz right as
            # the real stream begins.  The warm matmuls pre-accumulate
            # zeros into pair 0's real PSUM chains (start=True writes 0s,
            # the real taps then accumulate on top), so every PSUM write is
            # consumed by the normal copy path — a standalone never-read
            # warm PSUM tile hung the device (exec-unit timeout).
            if WARMUP:
                wtile = kpool.tile([128, 640], mybir.dt.float16, tag="warm_in")
                nc.gpsimd.memset(wtile[:, :], 0.0)
            # Loads go on the Scalar-engine HWDGE queue, stores on the Sync
            # queue — two independent FIFOs so the 8 MB of output stores never
            # serialize behind input loads.
            load_eng = nc.gpsimd if MM_DTYPE == "bf16" else nc.scalar

            x_dt = mybir.dt.bfloat16 if MM_DTYPE == "bf16" else in_dt

            # Pair 0 first, split into two row-halves so the first matmuls
            # only wait on 313 KB.  The kernel tile rides the (otherwise idle)
            # Sync queue concurrently.
            xtiles = []
            xtile0 = xpool.tile([128, HP, WP], x_dt, tag="x")
            load_eng.dma_start(out=xtile0[:, 0:18, :].rearrange("p h w -> p (h w)"),
                               in_=xp[0, :, 0:18 * WP])
            ktile = kpool.tile([128, KD * KD, F],
                               mybir.dt.bfloat16 if MM_DTYPE == "bf16" else k_dt)
            if MM_DTYPE == "bf16":
                nc.gpsimd.dma_start(out=ktile.rearrange("p t f -> p (t f)"), in_=kd[:, :])
                nc.gpsimd.dma_start(out=xtile0[:, 18:HP, :].rearrange("p h w -> p (h w)"),
                                    in_=xp[0, :, 18 * WP:HP * WP])
            else:
                # kernel tile on the (otherwise idle at startup) Sync queue,
                # concurrent with pair-0's load on the Scalar queue.  Tap-0
                # weights (32 KB) ride a separate first DMA so the first
                # matmul's dependency lands ~0.6us earlier than the full
                # 295 KB kernel tile.
                nc.sync.dma_start(out=ktile[:, 0, :], in_=kd[:, 0:F])
                # Remainder in two chunks: at warm-clock speed tap 1 fires
                # ~10.1us and tap 5 ~10.9us; a single 262KB DMA would land
                # ~10.2us with no margin, the split lands 65KB at ~9.7us.
                nc.sync.dma_start(
                    out=ktile[:, 1:5, :].rearrange("p t f -> p (t f)"),
                    in_=kd[:, F:5 * F])
                nc.sync.dma_start(
                    out=ktile[:, 5:KD * KD, :].rearrange("p t f -> p (t f)"),
                    in_=kd[:, 5 * F:KD * KD * F])
                load_eng.dma_start(out=xtile0[:, 18:HP, :].rearrange("p h w -> p (h w)"),
                                   in_=xp[0, :, 18 * WP:HP * WP])
            xtiles.append(xtile0)

            for pair in range(1, I // 2):
                xt = xpool.tile([128, HP, WP], x_dt, name=f"x_{pair}", tag="x")
                load_eng.dma_start(out=xt.rearrange("p h w -> p (h w)"), in_=xp[pair])
                xtiles.append(xt)

            def emit_mm(psums, xtile, schedule, warmed=False):
                # schedule: list of (half, par, t)
                for half, par, t in schedule:
                    kh, kw = divmod(t, KD)
                    oh0 = half * ROWS_PER_TILE
                    p0 = par * 64
                    lhsT = ktile[p0:p0 + 64, t, :]
                    rhs = xtile[p0:p0 + 64, oh0 + kh:oh0 + kh + ROWS_PER_TILE,
                                kw:kw + W]
                    nc.tensor.matmul(
                        psums[half][par][:, :], lhsT, rhs,
                        start=(t == 0 and not warmed), stop=(t == KD * KD - 1),
                    )

            for pair in range(I // 2):
                xtile = xtiles[pair]
                psums = []
                for half in range(NHALF):
                    row = []
                    for par in range(2):
                        ps = psum_pool.tile([128, NTILE], mybir.dt.float32,
                                            name=f"ps_{pair}_{half}_{par}", tag="ps")
                        row.append(ps)
                    psums.append(row)

                if pair == 0:
                    if WARMUP:
                        # Zero-valued warm matmuls rotating over the 4 real
                        # chains (start=True on each chain's first, so the
                        # chain begins as explicit zeros; real taps then
                        # accumulate with start=False).  Row groups alternate
                        # like the real tap-major schedule.
                        for i in range(N_WARM):
                            h, par = divmod(i % 4, 2)
                            p0 = par * 64
                            nc.tensor.matmul(
                                psums[h][par][:, :],
                                wtile[p0:p0 + 64, 0:128],
                                wtile[p0:p0 + 64, 128:640],
                                start=(i < 4), stop=False,
                            )
                    # half-major: half 0 only needs the first row-split load
                    sched = [(h, par, t) for h in range(NHALF)
                             for t in range(KD * KD) for par in range(2)]
                elif pair == I // 2 - 1:
                    # Last pair: taps 0-5 tap-major, then each chain finishes
                    # its last 3 taps as a trio.  Chains alternate PE
                    # row-groups so consecutive trios still overlap, but the
                    # completions spread ~0.6us apart — the DVE copies (681ns
                    # each) then run under the trailing matmuls instead of
                    # serializing after the final one.
                    sched = [(h, par, t) for t in range(KD * KD - 4)
                             for h in range(NHALF) for par in range(2)]
                    for h, par in ((0, 0), (0, 1), (1, 0), (1, 1)):
                        sched += [(h, par, t) for t in range(KD * KD - 4, KD * KD)]
                else:
                    # Taps 0-6 tap-major over all 4 psum chains (maximizes
                    # independent work in the PE queue so LDWEIGHTS stays
                    # hidden); the final two taps chain-grouped so the 4
                    # chains *finish* staggered and their PSUM->SBUF copies
                    # overlap the remaining matmuls instead of piling up
                    # after the last one.
                    sched = [(h, par, t) for t in range(KD * KD - 2)
                             for h in range(NHALF) for par in range(2)]
                    sched += [(h, par, t) for h in range(NHALF)
                              for par in range(2)
                              for t in (KD * KD - 2, KD * KD - 1)]
                emit_mm(psums, xtile, sched, warmed=(pair == 0 and WARMUP))

                # Stores alternate between the Sync and Scalar HWDGE queues:
                # each DMA_DIRECT2D issue costs ~600ns on its engine, so
                # pairing them across two engines halves the issue
                # serialization (the Scalar engine has issued all its input
                # loads by ~10.4us, well before the first store at ~13us).
                for half in range(NHALF):
                    for par in range(2):
                        i_img = pair * 2 + par
                        st_idx = pair * 4 + half * 2 + par
                        st_eng = nc.sync if st_idx % 2 == 0 else nc.scalar
                        otile = opool.tile([128, NTILE], out_dt,
                                           name=f"o_{pair}_{half}_{par}", tag="o")
                        if pair == I // 2 - 1:
                            # Last pair: split copy+store in half so the
                            # final store's transfer starts ~0.3us earlier.
                            for ch in range(2):
                                sl = slice(ch * (NTILE // 2), (ch + 1) * (NTILE // 2))
                                nc.vector.tensor_copy(out=otile[:, sl],
                                                      in_=psums[half][par][:, sl])
                                eng = nc.sync if (st_idx * 2 + ch) % 2 == 0 else nc.scalar
                                eng.dma_start(
                                    out=out[i_img, :,
                                            half * NTILE + ch * (NTILE // 2):
                                            half * NTILE + (ch + 1) * (NTILE // 2)],
                                    in_=otile[:, sl],
                                )
                        else:
                            nc.vector.tensor_copy(out=otile[:, :],
                                                  in_=psums[half][par][:, :])
                            st_eng.dma_start(
                                out=out[i_img, :, half * NTILE:(half + 1) * NTILE],
                                in_=otile[:, :],
                            )
    nc.compile()
    return nc


def _prep_core_inputs(x_b: np.ndarray, k_b: np.ndarray):
    """x_b (16,32,32,64) f32, k_b (3,3,64,128) f32 -> device layouts."""
    np_in = np.float16 if MM_DTYPE == "f16" else np.float32
    xpad = np.zeros((I, HP, WP, C), dtype=np_in)
    xpad[:, 1:H + 1, 1:W + 1, :] = x_b
    # (I, HP, WP, C) -> (I, C, HP, WP) -> (I//2, 2*C, HP*WP)
    xp = np.ascontiguousarray(xpad.transpose(0, 3, 1, 2)).reshape(I // 2, 2 * C, HP * WP)

    kc = k_b.reshape(KD * KD, C, F)                       # (9, 64, 128)
    kdup = np.concatenate([kc, kc], axis=1)               # (9, 128, 128)
    kd = np.ascontiguousarray(kdup.transpose(1, 0, 2)).reshape(128, KD * KD * F)
    if W_DTYPE == "f16" or MM_DTYPE == "f16":
        kd = kd.astype(np.float16)
    return {"xp": xp, "kd": kd}


def kernel(**inputs) -> np.ndarray:
    global _CACHED_NC, LAST_RESULTS
    x = np.asarray(inputs["x"], dtype=np.float32)
    k = np.asarray(inputs["kernel"], dtype=np.float32)

    if _CACHED_NC is None:
        _CACHED_NC = _build_nc()
    nc = _CACHED_NC

    in_maps = [_prep_core_inputs(x[b], k[b]) for b in range(B)]
    res = run_bass_kernel_spmd(nc, in_maps, core_ids=list(range(N_CORES)))
    LAST_RESULTS = res

    outs = []
    for b in range(B):
        o = np.asarray(res.results[b]["out"], dtype=np.float32)  # (16, 128, 1024)
        o = o.transpose(0, 2, 1).reshape(I, H, W, F)       # (16, 32, 32, 128)
        outs.append(o)
    return np.ascontiguousarray(np.stack(outs, axis=0))



# revision 19
# speedup vs baseline: 1.0463x; 1.0463x over previous
"""Batched conv layer (im2col gather + einsum) as a Bass/Tile TRN2 kernel.

Problem: x (8,16,32,32,64) f32, kernel (8,3,3,64,128) f32
         out[b,i,oh,ow,f] = sum_{kh,kw,c} xpad[b,i,oh+kh-1,ow+kw-1,c] * kernel[b,kh,kw,c,f]
         out (8,16,32,32,128) f32

Sharding: batch dim b across 8 cores (pure data parallel, no collectives).

Per-core device layout (host prepares these):
  xp : (8 pairs, 128, 34*34) f32   partition dim packs 2 images x 64 channels;
                                   free dim is the zero-padded 34x34 image plane
  kd : (128, 9*128) f32            partition dim packs 2 copies of the 64 channels
                                   (one per image in a pair); free dim is
                                   9 taps x 128 output filters
  out: (16, 128, 1024) f32         [image, filter, position]; host transposes back

The conv is computed as 9 shifted matmuls accumulated in PSUM:
  out[f, pos] += ktap[c, f].T @ xwin[c, pos]   for each tap (kh, kw)
Images are processed in pairs occupying PE row-groups 0-63 / 64-127 so two
K=64 matmuls can run concurrently in the 128x128 array.
"""

import os

import numpy as np

import concourse.bass as bass
import concourse.mybir as mybir
from concourse import bacc
from concourse.bass_utils import run_bass_kernel_spmd
from concourse.tile import TileContext

# Static problem config (hardcoded per the harness contract)
B, I, H, W, C, F = 8, 16, 32, 32, 64, 128
KD = 3
HP = H + 2  # padded
WP = W + 2
NPOS = H * W          # 1024 output positions per image
NTILE = 512           # positions per PSUM tile (one bank)
NHALF = NPOS // NTILE  # 2
ROWS_PER_TILE = NTILE // W  # 16 output rows per tile
N_CORES = 8

# matmul input dtype: "f16" (default: ~3e-4 rel err, fastest), "f32r"
# (~1.5e-4), "f32" (exact, 4x slower PE), "bf16"
MM_DTYPE = os.environ.get("CONV_MM_DTYPE", "f16")
# weight (stationary operand) dtype: "" = same as MM_DTYPE
W_DTYPE = os.environ.get("CONV_W_DTYPE", "")
# store outputs as f16 (host casts back to f32): halves the 8.4 MB of
# output HBM traffic and the PSUM->SBUF copy time.  Quantization adds
# ~3e-4 rel err on top of the f16-matmul ~3e-4 — far under the 2e-2 gate.
OUT_F16 = os.environ.get("CONV_OUT_F16", "1") == "1"
# PE warm-up matmuls (see below); "1" = on.
WARMUP = os.environ.get("CONV_WARMUP", "1") == "1"
N_WARM = max(4, int(os.environ.get("CONV_N_WARM", "7")))

_CACHED_NC = None
LAST_RESULTS = None


def _build_nc():
    nc = bacc.Bacc(trn_type="TRN2")

    mm_dt = {
        "f32": mybir.dt.float32,
        "f32r": mybir.dt.float32r,
        "bf16": mybir.dt.bfloat16,
        "f16": mybir.dt.float16,
    }[MM_DTYPE]
    # For f32r, type the DRAM inputs as float32r end-to-end (same 4-byte fp32
    # layout; the PE just reads fewer mantissa bits) so the BIR verifier sees a
    # consistent fp32r producer chain.  For f16 the host pre-casts the inputs.
    if MM_DTYPE in ("f32r", "f16"):
        in_dt = mm_dt
    else:
        in_dt = mybir.dt.float32

    k_dt = mybir.dt.float16 if W_DTYPE == "f16" else in_dt

    out_dt = mybir.dt.float16 if OUT_F16 else mybir.dt.float32

    xp = nc.declare_dram_parameter("xp", [I // 2, 128, HP * WP], in_dt, isOutput=False)
    kd = nc.declare_dram_parameter("kd", [128, KD * KD * F], k_dt, isOutput=False)
    out = nc.declare_dram_parameter("out", [I, F, NPOS], out_dt, isOutput=True)

    with TileContext(nc) as tc:
        with (
            tc.tile_pool(name="kpool", bufs=1) as kpool,
            tc.tile_pool(name="xpool", bufs=8) as xpool,
            tc.tile_pool(name="opool", bufs=32) as opool,
            tc.tile_pool(name="psum", bufs=8, space="PSUM") as psum_pool,
        ):
            # PE warm-up: the HAM clock gate runs the PE at 1.2 GHz until it
            # has seen ~3.4us of sustained activity; the first real matmul
            # can't start before ~9.8us (framework preamble ~6.8us + first
            # input DMA ~3us).  Burn zero-valued matmuls on a memset tile
            # during that dead window so the HAM flips to 2.4 GHz right as
            # the real stream begins.  The warm matmuls pre-accumulate
            # zeros into pair 0's real PSUM chains (start=True writes 0s,
            # the real taps then accumulate on top), so every PSUM write is
            # consumed by the normal copy path — a standalone never-read
            # warm PSUM tile hung the device (exec-unit timeout).
            if WARMUP:
                wtile = kpool.tile([128, 640], mybir.dt.float16, tag="warm_in")
                nc.vector.memset(wtile[:, :], 0.0)
            # Loads go on the Scalar-engine HWDGE queue, stores on the Sync
            # queue — two independent FIFOs so the 8 MB of output stores never
            # serialize behind input loads.
            load_eng = nc.gpsimd if MM_DTYPE == "bf16" else nc.scalar

            x_dt = mybir.dt.bfloat16 if MM_DTYPE == "bf16" else in_dt

            # One full-row DMA per transfer: sub-row slices of the DRAM
            # params are strided (1-2KB descriptors) and run at ~64 GB/s vs
            # ~340 GB/s for full contiguous rows.  The kernel tile (295 KB,
            # full rows) rides the otherwise-idle Sync queue and lands
            # ~9.7us; the PE is busy with warm-up matmuls until ~10.4us, so
            # nothing stalls on it.
            ktile = kpool.tile([128, KD * KD, F],
                               mybir.dt.bfloat16 if MM_DTYPE == "bf16" else k_dt)
            k_eng = nc.gpsimd if MM_DTYPE == "bf16" else nc.sync
            k_eng.dma_start(out=ktile.rearrange("p t f -> p (t f)"), in_=kd[:, :])
            xtiles = []
            for pair in range(I // 2):
                xt = xpool.tile([128, HP, WP], x_dt, name=f"x_{pair}", tag="x")
                load_eng.dma_start(out=xt.rearrange("p h w -> p (h w)"), in_=xp[pair])
                xtiles.append(xt)

            def emit_mm(psums, xtile, schedule, warmed=False):
                # schedule: list of (half, par, t)
                for half, par, t in schedule:
                    kh, kw = divmod(t, KD)
                    oh0 = half * ROWS_PER_TILE
                    p0 = par * 64
                    lhsT = ktile[p0:p0 + 64, t, :]
                    rhs = xtile[p0:p0 + 64, oh0 + kh:oh0 + kh + ROWS_PER_TILE,
                                kw:kw + W]
                    nc.tensor.matmul(
                        psums[half][par][:, :], lhsT, rhs,
                        start=(t == 0 and not warmed), stop=(t == KD * KD - 1),
                    )

            for pair in range(I // 2):
                xtile = xtiles[pair]
                psums = []
                for half in range(NHALF):
                    row = []
                    for par in range(2):
                        ps = psum_pool.tile([128, NTILE], mybir.dt.float32,
                                            name=f"ps_{pair}_{half}_{par}", tag="ps")
                        row.append(ps)
                    psums.append(row)

                if pair == 0:
                    if WARMUP:
                        # Zero-valued warm matmuls rotating over the 4 real
                        # chains (start=True on each chain's first, so the
                        # chain begins as explicit zeros; real taps then
                        # accumulate with start=False).  All warm matmuls use
                        # row group h0 so they SERIALIZE in the PE (two
                        # row-group-disjoint matmuls run concurrently, which
                        # would halve the busy window the warm-up exists to
                        # fill): N x 427ns of continuous PE activity from
                        # ~7.2us until the first input DMA lands ~10us.
                        for i in range(N_WARM):
                            h, par = divmod(i % 4, 2)
                            nc.tensor.matmul(
                                psums[h][par][:, :],
                                wtile[0:64, 0:128],
                                wtile[0:64, 128:640],
                                start=(i < 4), stop=False,
                            )
                    sched = [(h, par, t) for t in range(KD * KD - 2)
                             for h in range(NHALF) for par in range(2)]
                    sched += [(h, par, t) for h in range(NHALF)
                              for par in range(2)
                              for t in (KD * KD - 2, KD * KD - 1)]
                elif pair == I // 2 - 1:
                    # Last pair: each 512-col chain is split into two
                    # independent 256-col accumulation groups, processed as
                    # four staggered duos (par0+par1 concurrent).  Each duo's
                    # outputs copy+store while the next duo's matmuls run, so
                    # after the very last matmul only a 256-col copy + 65KB
                    # store remain (~2.5us tail instead of ~5us).
                    for h, cg in ((0, 0), (1, 0), (0, 1), (1, 1)):
                        c0 = cg * (NTILE // 2)
                        r0 = h * ROWS_PER_TILE + cg * (ROWS_PER_TILE // 2)
                        for t in range(KD * KD):
                            kh, kw = divmod(t, KD)
                            for par in range(2):
                                p0 = par * 64
                                nc.tensor.matmul(
                                    psums[h][par][:, c0:c0 + NTILE // 2],
                                    ktile[p0:p0 + 64, t, :],
                                    xtile[p0:p0 + 64,
                                          r0 + kh:r0 + kh + ROWS_PER_TILE // 2,
                                          kw:kw + W],
                                    start=(t == 0), stop=(t == KD * KD - 1),
                                )
                        for par in range(2):
                            i_img = pair * 2 + par
                            otile = opool.tile([128, NTILE // 2], out_dt,
                                               name=f"og_{h}_{cg}_{par}", tag="o")
                            nc.vector.tensor_copy(
                                out=otile[:, :],
                                in_=psums[h][par][:, c0:c0 + NTILE // 2])
                            eng = nc.sync if par == 0 else nc.scalar
                            eng.dma_start(
                                out=out[i_img, :,
                                        h * NTILE + c0:h * NTILE + c0 + NTILE // 2],
                                in_=otile[:, :],
                            )
                    continue
                else:
                    # Taps 0-6 tap-major over all 4 psum chains (maximizes
                    # independent work in the PE queue so LDWEIGHTS stays
                    # hidden); the final two taps chain-grouped so the 4
                    # chains *finish* staggered and their PSUM->SBUF copies
                    # overlap the remaining matmuls instead of piling up
                    # after the last one.
                    sched = [(h, par, t) for t in range(KD * KD - 2)
                             for h in range(NHALF) for par in range(2)]
                    sched += [(h, par, t) for h in range(NHALF)
                              for par in range(2)
                              for t in (KD * KD - 2, KD * KD - 1)]
                emit_mm(psums, xtile, sched, warmed=(pair == 0 and WARMUP))

                # Stores alternate between the Sync and Scalar HWDGE queues:
                # each DMA_DIRECT2D issue costs ~600ns on its engine, so
                # pairing them across two engines halves the issue
                # serialization (the Scalar engine has issued all its input
                # loads by ~10.4us, well before the first store at ~13us).
                for half in range(NHALF):
                    for par in range(2):
                        i_img = pair * 2 + par
                        st_idx = pair * 4 + half * 2 + par
                        st_eng = nc.sync if st_idx % 2 == 0 else nc.scalar
                        otile = opool.tile([128, NTILE], out_dt,
                                           name=f"o_{pair}_{half}_{par}", tag="o")
                        nc.vector.tensor_copy(out=otile[:, :],
                                              in_=psums[half][par][:, :])
                        st_eng.dma_start(
                            out=out[i_img, :, half * NTILE:(half + 1) * NTILE],
                            in_=otile[:, :],
                        )
    nc.compile()
    return nc


def _prep_core_inputs(x_b: np.ndarray, k_b: np.ndarray):
    """x_b (16,32,32,64) f32, k_b (3,3,64,128) f32 -> device layouts."""
    np_in = np.float16 if MM_DTYPE == "f16" else np.float32
    xpad = np.zeros((I, HP, WP, C), dtype=np_in)
    xpad[:, 1:H + 1, 1:W + 1, :] = x_b
    # (I, HP, WP, C) -> (I, C, HP, WP) -> (I//2, 2*C, HP*WP)
    xp = np.ascontiguousarray(xpad.transpose(0, 3, 1, 2)).reshape(I // 2, 2 * C, HP * WP)

    kc = k_b.reshape(KD * KD, C, F)                       # (9, 64, 128)
    kdup = np.concatenate([kc, kc], axis=1)               # (9, 128, 128)
    kd = np.ascontiguousarray(kdup.transpose(1, 0, 2)).reshape(128, KD * KD * F)
    if W_DTYPE == "f16" or MM_DTYPE == "f16":
        kd = kd.astype(np.float16)
    return {"xp": xp, "kd": kd}


def kernel(**inputs) -> np.ndarray:
    global _CACHED_NC, LAST_RESULTS
    x = np.asarray(inputs["x"], dtype=np.float32)
    k = np.asarray(inputs["kernel"], dtype=np.float32)

    if _CACHED_NC is None:
        _CACHED_NC = _build_nc()
    nc = _CACHED_NC

    in_maps = [_prep_core_inputs(x[b], k[b]) for b in range(B)]
    res = run_bass_kernel_spmd(nc, in_maps, core_ids=list(range(N_CORES)))
    LAST_RESULTS = res

    outs = []
    for b in range(B):
        o = np.asarray(res.results[b]["out"], dtype=np.float32)  # (16, 128, 1024)
        o = o.transpose(0, 2, 1).reshape(I, H, W, F)       # (16, 32, 32, 128)
        outs.append(o)
    return np.ascontiguousarray(np.stack(outs, axis=0))



# revision 32
# speedup vs baseline: 1.0474x; 1.0010x over previous
"""Batched conv layer (im2col gather + einsum) as a Bass/Tile TRN2 kernel.

Problem: x (8,16,32,32,64) f32, kernel (8,3,3,64,128) f32
         out[b,i,oh,ow,f] = sum_{kh,kw,c} xpad[b,i,oh+kh-1,ow+kw-1,c] * kernel[b,kh,kw,c,f]
         out (8,16,32,32,128) f32

Sharding: batch dim b across 8 cores (pure data parallel, no collectives).

Per-core device layout (host prepares these):
  xp : (8 pairs, 128, 34*34) f32   partition dim packs 2 images x 64 channels;
                                   free dim is the zero-padded 34x34 image plane
  kd : (128, 9*128) f32            partition dim packs 2 copies of the 64 channels
                                   (one per image in a pair); free dim is
                                   9 taps x 128 output filters
  out: (16, 128, 1024) f32         [image, filter, position]; host transposes back

The conv is computed as 9 shifted matmuls accumulated in PSUM:
  out[f, pos] += ktap[c, f].T @ xwin[c, pos]   for each tap (kh, kw)
Images are processed in pairs occupying PE row-groups 0-63 / 64-127 so two
K=64 matmuls can run concurrently in the 128x128 array.
"""

import os

import numpy as np

import concourse.bass as bass
import concourse.mybir as mybir
from concourse import bacc
from concourse.bass_utils import run_bass_kernel_spmd
from concourse.tile import TileContext

# Static problem config (hardcoded per the harness contract)
B, I, H, W, C, F = 8, 16, 32, 32, 64, 128
KD = 3
HP = H + 2  # padded
WP = W + 2
NPOS = H * W          # 1024 output positions per image
NTILE = 512           # positions per PSUM tile (one bank)
NHALF = NPOS // NTILE  # 2
ROWS_PER_TILE = NTILE // W  # 16 output rows per tile
N_CORES = 8

# matmul input dtype: "f16" (default: ~3e-4 rel err, fastest), "f32r"
# (~1.5e-4), "f32" (exact, 4x slower PE), "bf16"
MM_DTYPE = os.environ.get("CONV_MM_DTYPE", "f16")
# weight (stationary operand) dtype: "" = same as MM_DTYPE
W_DTYPE = os.environ.get("CONV_W_DTYPE", "")
# store outputs as f16 (host casts back to f32): halves the 8.4 MB of
# output HBM traffic and the PSUM->SBUF copy time.  Quantization adds
# ~3e-4 rel err on top of the f16-matmul ~3e-4 — far under the 2e-2 gate.
OUT_F16 = os.environ.get("CONV_OUT_F16", "1") == "1"
# PE warm-up matmuls (see below); "1" = on.
WARMUP = os.environ.get("CONV_WARMUP", "1") == "1"
N_WARM = max(4, int(os.environ.get("CONV_N_WARM", "7")))

_CACHED_NC = None
LAST_RESULTS = None


def _build_nc():
    nc = bacc.Bacc(trn_type="TRN2")

    mm_dt = {
        "f32": mybir.dt.float32,
        "f32r": mybir.dt.float32r,
        "bf16": mybir.dt.bfloat16,
        "f16": mybir.dt.float16,
    }[MM_DTYPE]
    # For f32r, type the DRAM inputs as float32r end-to-end (same 4-byte fp32
    # layout; the PE just reads fewer mantissa bits) so the BIR verifier sees a
    # consistent fp32r producer chain.  For f16 the host pre-casts the inputs.
    if MM_DTYPE in ("f32r", "f16"):
        in_dt = mm_dt
    else:
        in_dt = mybir.dt.float32

    k_dt = mybir.dt.float16 if W_DTYPE == "f16" else in_dt

    out_dt = mybir.dt.float16 if OUT_F16 else mybir.dt.float32

    # Pair-0's x and the kernel tile are split into separate DRAM params so
    # every front-end DMA reads full contiguous param rows.  Sub-row strided
    # slices (e.g. kd[:, 0:F]) as DMA sources while warm-up matmuls run on
    # the PE hung the device reproducibly (exec-unit unrecoverable); with
    # full-row sources the warm-up runs clean.
    xp = nc.declare_dram_parameter("xp", [I // 2, 128, HP * WP], in_dt, isOutput=False)
    xp0a = nc.declare_dram_parameter("xp0a", [128, 18 * WP], in_dt, isOutput=False)
    xp0b = nc.declare_dram_parameter("xp0b", [128, (HP - 18) * WP], in_dt, isOutput=False)
    kdf = nc.declare_dram_parameter("kdf", [128, KD * KD * F], k_dt, isOutput=False)
    kd0 = nc.declare_dram_parameter("kd0", [128, F], k_dt, isOutput=False)
    kdR = nc.declare_dram_parameter("kdR", [128, (KD * KD - 1) * F], k_dt, isOutput=False)
    out = nc.declare_dram_parameter("out", [I, F, NPOS], out_dt, isOutput=True)

    with TileContext(nc) as tc:
        with (
            tc.tile_pool(name="kpool", bufs=1) as kpool,
            tc.tile_pool(name="xpool", bufs=8) as xpool,
            tc.tile_pool(name="opool", bufs=32) as opool,
            tc.tile_pool(name="psum", bufs=8, space="PSUM") as psum_pool,
        ):
            # PE warm-up: the HAM clock gate runs the PE at 1.2 GHz until it
            # has seen ~3.4us of sustained activity; the first real matmul
            # can't start before ~9.8us (framework preamble ~6.8us + first
            # input DMA ~3us).  Burn zero-valued matmuls on a memset tile
            # during that dead window so the HAM flips to 2.4 GHz right as
            # the real stream begins.  The warm matmuls pre-accumulate
            # zeros into pair 0's real PSUM chains (start=True writes 0s,
            # the real taps then accumulate on top), so every PSUM write is
            # consumed by the normal copy path — a standalone never-read
            # warm PSUM tile hung the device (exec-unit timeout).
            if WARMUP:
                wtile = kpool.tile([128, 640], mybir.dt.float16, tag="warm_in")
                nc.vector.memset(wtile[:, :], 0.0)
            # Loads go on the Scalar-engine HWDGE queue, stores on the Sync
            # queue — two independent FIFOs so the 8 MB of output stores never
            # serialize behind input loads.
            load_eng = nc.gpsimd if MM_DTYPE == "bf16" else nc.scalar

            x_dt = mybir.dt.bfloat16 if MM_DTYPE == "bf16" else in_dt

            # Front-end DMA latency is the critical path to the first real
            # matmul (~9.8us: ~6.8us framework preamble + ~3us issue+land).
            # Keep the first dependencies SMALL: pair-0's x rides in two
            # row-halves on the Scalar queue while the kernel tile (tap-0
            # first) rides the otherwise-idle Sync queue.  A single big
            # 296KB x0 + 295KB kd pair of head-of-line transfers was
            # measured to push the first matmul out to 11.6us.
            # Queue plan (empirically constrained: with warm-up matmuls on
            # the PE, a 9th load-DMA on the Scalar queue hangs the device —
            # its engine queue is 8 deep — and pair-0's x must stay a
            # single DMA):
            #   Sync   : x0 (296 KB, the first matmul's critical dep) alone,
            #            so it shares its queue with nothing; then stores.
            #   Scalar : kdR first (taps 1-8, needed ~0.2us after tap 0),
            #            then x1..x7  (8 loads total); then stores.
            #   GpSimd : kd0 (32 KB, SWDGE) — lands ~9.3us, before x0.
            xtiles = []
            xtile0 = xpool.tile([128, HP, WP], x_dt, tag="x")
            ktile = kpool.tile([128, KD * KD, F],
                               mybir.dt.bfloat16 if MM_DTYPE == "bf16" else k_dt)
            nc.sync.dma_start(out=xtile0.rearrange("p h w -> p (h w)"), in_=xp[0])
            nc.gpsimd.dma_start(out=ktile[:, 0, :], in_=kd0[:, :])
            load_eng.dma_start(
                out=ktile[:, 1:KD * KD, :].rearrange("p t f -> p (t f)"),
                in_=kdR[:, :])
            xtiles.append(xtile0)
            for pair in range(1, I // 2):
                xt = xpool.tile([128, HP, WP], x_dt, name=f"x_{pair}", tag="x")
                load_eng.dma_start(out=xt.rearrange("p h w -> p (h w)"), in_=xp[pair])
                xtiles.append(xt)

            def emit_mm(psums, xtile, schedule, warmed=False):
                # schedule: list of (half, par, t)
                for half, par, t in schedule:
                    kh, kw = divmod(t, KD)
                    oh0 = half * ROWS_PER_TILE
                    p0 = par * 64
                    lhsT = ktile[p0:p0 + 64, t, :]
                    rhs = xtile[p0:p0 + 64, oh0 + kh:oh0 + kh + ROWS_PER_TILE,
                                kw:kw + W]
                    nc.tensor.matmul(
                        psums[half][par][:, :], lhsT, rhs,
                        start=(t == 0 and not warmed), stop=(t == KD * KD - 1),
                    )

            for pair in range(I // 2):
                xtile = xtiles[pair]
                psums = []
                for half in range(NHALF):
                    row = []
                    for par in range(2):
                        ps = psum_pool.tile([128, NTILE], mybir.dt.float32,
                                            name=f"ps_{pair}_{half}_{par}", tag="ps")
                        row.append(ps)
                    psums.append(row)

                if pair == 0:
                    if WARMUP:
                        # Zero-valued warm matmuls rotating over the 4 real
                        # chains (start=True on each chain's first, so the
                        # chain begins as explicit zeros; real taps then
                        # accumulate with start=False).  All warm matmuls use
                        # row group h0 so they SERIALIZE in the PE (two
                        # row-group-disjoint matmuls run concurrently, which
                        # would halve the busy window the warm-up exists to
                        # fill): N x 427ns of continuous PE activity from
                        # ~7.2us until the first input DMA lands ~10us.
                        for i in range(N_WARM):
                            h, par = divmod(i % 4, 2)
                            nc.tensor.matmul(
                                psums[h][par][:, :],
                                wtile[0:64, 0:128],
                                wtile[0:64, 128:640],
                                start=(i < 4), stop=False,
                            )
                    # x0 arrives as one DMA, so both halves are ready at
                    # once — use the same tap-major + staggered-finish
                    # schedule as the middle pairs.
                    sched = [(h, par, t) for t in range(KD * KD - 2)
                             for h in range(NHALF) for par in range(2)]
                    sched += [(h, par, t) for h in range(NHALF)
                              for par in range(2)
                              for t in (KD * KD - 2, KD * KD - 1)]
                elif pair == I // 2 - 1:
                    # Last pair: each 512-col chain is split into two
                    # independent 256-col accumulation groups, processed as
                    # four staggered duos (par0+par1 concurrent).  Each duo's
                    # outputs copy+store while the next duo's matmuls run, so
                    # after the very last matmul only a 256-col copy + 65KB
                    # store remain (~2.5us tail instead of ~5us).
                    for h, cg in ((0, 0), (1, 0), (0, 1), (1, 1)):
                        c0 = cg * (NTILE // 2)
                        r0 = h * ROWS_PER_TILE + cg * (ROWS_PER_TILE // 2)
                        for t in range(KD * KD):
                            kh, kw = divmod(t, KD)
                            for par in range(2):
                                p0 = par * 64
                                nc.tensor.matmul(
                                    psums[h][par][:, c0:c0 + NTILE // 2],
                                    ktile[p0:p0 + 64, t, :],
                                    xtile[p0:p0 + 64,
                                          r0 + kh:r0 + kh + ROWS_PER_TILE // 2,
                                          kw:kw + W],
                                    start=(t == 0), stop=(t == KD * KD - 1),
                                )
                        for par in range(2):
                            i_img = pair * 2 + par
                            otile = opool.tile([128, NTILE // 2], out_dt,
                                               name=f"og_{h}_{cg}_{par}", tag="o")
                            nc.vector.tensor_copy(
                                out=otile[:, :],
                                in_=psums[h][par][:, c0:c0 + NTILE // 2])
                            eng = nc.sync if par == 0 else nc.scalar
                            eng.dma_start(
                                out=out[i_img, :,
                                        h * NTILE + c0:h * NTILE + c0 + NTILE // 2],
                                in_=otile[:, :],
                            )
                    continue
                else:
                    # Taps 0-6 tap-major over all 4 psum chains (maximizes
                    # independent work in the PE queue so LDWEIGHTS stays
                    # hidden); the final two taps chain-grouped so the 4
                    # chains *finish* staggered and their PSUM->SBUF copies
                    # overlap the remaining matmuls instead of piling up
                    # after the last one.
                    sched = [(h, par, t) for t in range(KD * KD - 2)
                             for h in range(NHALF) for par in range(2)]
                    sched += [(h, par, t) for h in range(NHALF)
                              for par in range(2)
                              for t in (KD * KD - 2, KD * KD - 1)]
                emit_mm(psums, xtile, sched, warmed=(pair == 0 and WARMUP))

                # Stores alternate between the Sync and Scalar HWDGE queues:
                # each DMA_DIRECT2D issue costs ~600ns on its engine, so
                # pairing them across two engines halves the issue
                # serialization (the Scalar engine has issued all its input
                # loads by ~10.4us, well before the first store at ~13us).
                for half in range(NHALF):
                    for par in range(2):
                        i_img = pair * 2 + par
                        st_idx = pair * 4 + half * 2 + par
                        st_eng = nc.sync if st_idx % 2 == 0 else nc.scalar
                        otile = opool.tile([128, NTILE], out_dt,
                                           name=f"o_{pair}_{half}_{par}", tag="o")
                        nc.vector.tensor_copy(out=otile[:, :],
                                              in_=psums[half][par][:, :])
                        st_eng.dma_start(
                            out=out[i_img, :, half * NTILE:(half + 1) * NTILE],
                            in_=otile[:, :],
                        )
    nc.compile()
    return nc


def _prep_core_inputs(x_b: np.ndarray, k_b: np.ndarray):
    """x_b (16,32,32,64) f32, k_b (3,3,64,128) f32 -> device layouts."""
    np_in = np.float16 if MM_DTYPE == "f16" else np.float32
    xpad = np.zeros((I, HP, WP, C), dtype=np_in)
    xpad[:, 1:H + 1, 1:W + 1, :] = x_b
    # (I, HP, WP, C) -> (I, C, HP, WP) -> (I//2, 2*C, HP*WP)
    xp = np.ascontiguousarray(xpad.transpose(0, 3, 1, 2)).reshape(I // 2, 2 * C, HP * WP)

    kc = k_b.reshape(KD * KD, C, F)                       # (9, 64, 128)
    kdup = np.concatenate([kc, kc], axis=1)               # (9, 128, 128)
    kd = np.ascontiguousarray(kdup.transpose(1, 0, 2)).reshape(128, KD * KD * F)
    if W_DTYPE == "f16" or MM_DTYPE == "f16":
        kd = kd.astype(np.float16)
    return {
        "xp": xp,
        "xp0a": np.ascontiguousarray(xp[0][:, :18 * WP]),
        "xp0b": np.ascontiguousarray(xp[0][:, 18 * WP:]),
        "kdf": kd,
        "kd0": np.ascontiguousarray(kd[:, :F]),
        "kdR": np.ascontiguousarray(kd[:, F:]),
    }


def kernel(**inputs) -> np.ndarray:
    global _CACHED_NC, LAST_RESULTS
    x = np.asarray(inputs["x"], dtype=np.float32)
    k = np.asarray(inputs["kernel"], dtype=np.float32)

    if _CACHED_NC is None:
        _CACHED_NC = _build_nc()
    nc = _CACHED_NC

    in_maps = [_prep_core_inputs(x[b], k[b]) for b in range(B)]
    res = run_bass_kernel_spmd(nc, in_maps, core_ids=list(range(N_CORES)))
    LAST_RESULTS = res

    outs = []
    for b in range(B):
        o = np.asarray(res.results[b]["out"], dtype=np.float32)  # (16, 128, 1024)
        o = o.transpose(0, 2, 1).reshape(I, H, W, F)       # (16, 32, 32, 128)
        outs.append(o)
    return np.ascontiguousarray(np.stack(outs, axis=0))



# revision 35
# speedup vs baseline: 1.0530x; 1.0054x over previous
"""Batched conv layer (im2col gather + einsum) as a Bass/Tile TRN2 kernel.

Problem: x (8,16,32,32,64) f32, kernel (8,3,3,64,128) f32
         out[b,i,oh,ow,f] = sum_{kh,kw,c} xpad[b,i,oh+kh-1,ow+kw-1,c] * kernel[b,kh,kw,c,f]
         out (8,16,32,32,128) f32

Sharding: batch dim b across 8 cores (pure data parallel, no collectives).

Per-core device layout (host prepares these):
  xp : (8 pairs, 128, 34*34) f32   partition dim packs 2 images x 64 channels;
                                   free dim is the zero-padded 34x34 image plane
  kd : (128, 9*128) f32            partition dim packs 2 copies of the 64 channels
                                   (one per image in a pair); free dim is
                                   9 taps x 128 output filters
  out: (16, 128, 1024) f32         [image, filter, position]; host transposes back

The conv is computed as 9 shifted matmuls accumulated in PSUM:
  out[f, pos] += ktap[c, f].T @ xwin[c, pos]   for each tap (kh, kw)
Images are processed in pairs occupying PE row-groups 0-63 / 64-127 so two
K=64 matmuls can run concurrently in the 128x128 array.
"""

import os

import numpy as np

import concourse.bass as bass
import concourse.mybir as mybir
from concourse import bacc
from concourse.bass_utils import run_bass_kernel_spmd
from concourse.tile import TileContext

# Static problem config (hardcoded per the harness contract)
B, I, H, W, C, F = 8, 16, 32, 32, 64, 128
KD = 3
HP = H + 2  # padded
WP = W + 2
NPOS = H * W          # 1024 output positions per image
NTILE = 512           # positions per PSUM tile (one bank)
NHALF = NPOS // NTILE  # 2
ROWS_PER_TILE = NTILE // W  # 16 output rows per tile
N_CORES = 8

# matmul input dtype: "f16" (default: ~3e-4 rel err, fastest), "f32r"
# (~1.5e-4), "f32" (exact, 4x slower PE), "bf16"
MM_DTYPE = os.environ.get("CONV_MM_DTYPE", "f16")
# weight (stationary operand) dtype: "" = same as MM_DTYPE
W_DTYPE = os.environ.get("CONV_W_DTYPE", "")
# store outputs as f16 (host casts back to f32): halves the 8.4 MB of
# output HBM traffic and the PSUM->SBUF copy time.  Quantization adds
# ~3e-4 rel err on top of the f16-matmul ~3e-4 — far under the 2e-2 gate.
OUT_F16 = os.environ.get("CONV_OUT_F16", "1") == "1"
# PE warm-up matmuls (see below); "1" = on.
WARMUP = os.environ.get("CONV_WARMUP", "1") == "1"
N_WARM = max(4, int(os.environ.get("CONV_N_WARM", "6")))

_CACHED_NC = None
LAST_RESULTS = None


def _build_nc():
    nc = bacc.Bacc(trn_type="TRN2")

    mm_dt = {
        "f32": mybir.dt.float32,
        "f32r": mybir.dt.float32r,
        "bf16": mybir.dt.bfloat16,
        "f16": mybir.dt.float16,
    }[MM_DTYPE]
    # For f32r, type the DRAM inputs as float32r end-to-end (same 4-byte fp32
    # layout; the PE just reads fewer mantissa bits) so the BIR verifier sees a
    # consistent fp32r producer chain.  For f16 the host pre-casts the inputs.
    if MM_DTYPE in ("f32r", "f16"):
        in_dt = mm_dt
    else:
        in_dt = mybir.dt.float32

    k_dt = mybir.dt.float16 if W_DTYPE == "f16" else in_dt

    out_dt = mybir.dt.float16 if OUT_F16 else mybir.dt.float32

    # Pair-0's x and the kernel tile are split into separate DRAM params so
    # every front-end DMA reads full contiguous param rows.  Sub-row strided
    # slices (e.g. kd[:, 0:F]) as DMA sources while warm-up matmuls run on
    # the PE hung the device reproducibly (exec-unit unrecoverable); with
    # full-row sources the warm-up runs clean.
    xp = nc.declare_dram_parameter("xp", [I // 2, 128, HP * WP], in_dt, isOutput=False)
    xp0a = nc.declare_dram_parameter("xp0a", [128, 18 * WP], in_dt, isOutput=False)
    xp0b = nc.declare_dram_parameter("xp0b", [128, (HP - 18) * WP], in_dt, isOutput=False)
    kdf = nc.declare_dram_parameter("kdf", [128, KD * KD * F], k_dt, isOutput=False)
    kd0 = nc.declare_dram_parameter("kd0", [128, F], k_dt, isOutput=False)
    kdR = nc.declare_dram_parameter("kdR", [128, (KD * KD - 1) * F], k_dt, isOutput=False)
    out = nc.declare_dram_parameter("out", [I, F, NPOS], out_dt, isOutput=True)

    with TileContext(nc) as tc:
        with (
            tc.tile_pool(name="kpool", bufs=1) as kpool,
            tc.tile_pool(name="xpool", bufs=8) as xpool,
            tc.tile_pool(name="opool", bufs=32) as opool,
            tc.tile_pool(name="psum", bufs=8, space="PSUM") as psum_pool,
        ):
            # PE warm-up: the HAM clock gate runs the PE at 1.2 GHz until it
            # has seen ~3.4us of sustained activity; the first real matmul
            # can't start before ~9.8us (framework preamble ~6.8us + first
            # input DMA ~3us).  Burn zero-valued matmuls on a memset tile
            # during that dead window so the HAM flips to 2.4 GHz right as
            # the real stream begins.  The warm matmuls pre-accumulate
            # zeros into pair 0's real PSUM chains (start=True writes 0s,
            # the real taps then accumulate on top), so every PSUM write is
            # consumed by the normal copy path — a standalone never-read
            # warm PSUM tile hung the device (exec-unit timeout).
            if WARMUP:
                wtile = kpool.tile([128, 640], mybir.dt.float16, tag="warm_in")
                nc.vector.memset(wtile[:, :], 0.0)
            # Loads go on the Scalar-engine HWDGE queue, stores on the Sync
            # queue — two independent FIFOs so the 8 MB of output stores never
            # serialize behind input loads.
            load_eng = nc.gpsimd if MM_DTYPE == "bf16" else nc.scalar

            x_dt = mybir.dt.bfloat16 if MM_DTYPE == "bf16" else in_dt

            # Front-end DMA latency is the critical path to the first real
            # matmul (~9.8us: ~6.8us framework preamble + ~3us issue+land).
            # Keep the first dependencies SMALL: pair-0's x rides in two
            # row-halves on the Scalar queue while the kernel tile (tap-0
            # first) rides the otherwise-idle Sync queue.  A single big
            # 296KB x0 + 295KB kd pair of head-of-line transfers was
            # measured to push the first matmul out to 11.6us.
            # Queue plan.  Empirical constraint: with warm-up matmuls on
            # the PE, a 9th load-DMA on the Scalar queue hangs the device
            # (its engine queue is 8 deep; every warm-up variant with >=9
            # scalar loads died NRT_EXEC_UNIT_UNRECOVERABLE, every one with
            # <=8 ran clean).
            #   Sync   : x0 rows 0-17 (157 KB, the first matmul's critical
            #            dep — lands ~9.8us), then rows 18-33; then stores.
            #   Scalar : kdR first (taps 1-8, needed ~0.2us after tap 0),
            #            then x1..x7  (8 loads total); then stores.
            #   GpSimd : kd0 (32 KB, SWDGE) — lands ~9.3us, before x0a.
            xtiles = []
            xtile0 = xpool.tile([128, HP, WP], x_dt, tag="x")
            ktile = kpool.tile([128, KD * KD, F],
                               mybir.dt.bfloat16 if MM_DTYPE == "bf16" else k_dt)
            nc.sync.dma_start(out=xtile0[:, 0:18, :].rearrange("p h w -> p (h w)"),
                              in_=xp0a[:, :])
            nc.sync.dma_start(out=xtile0[:, 18:HP, :].rearrange("p h w -> p (h w)"),
                              in_=xp0b[:, :])
            nc.gpsimd.dma_start(out=ktile[:, 0, :], in_=kd0[:, :])
            load_eng.dma_start(
                out=ktile[:, 1:KD * KD, :].rearrange("p t f -> p (t f)"),
                in_=kdR[:, :])
            xtiles.append(xtile0)
            for pair in range(1, I // 2):
                xt = xpool.tile([128, HP, WP], x_dt, name=f"x_{pair}", tag="x")
                load_eng.dma_start(out=xt.rearrange("p h w -> p (h w)"), in_=xp[pair])
                xtiles.append(xt)

            def emit_mm(psums, xtile, schedule, warmed=False):
                # schedule: list of (half, par, t)
                for half, par, t in schedule:
                    kh, kw = divmod(t, KD)
                    oh0 = half * ROWS_PER_TILE
                    p0 = par * 64
                    lhsT = ktile[p0:p0 + 64, t, :]
                    rhs = xtile[p0:p0 + 64, oh0 + kh:oh0 + kh + ROWS_PER_TILE,
                                kw:kw + W]
                    nc.tensor.matmul(
                        psums[half][par][:, :], lhsT, rhs,
                        start=(t == 0 and not warmed), stop=(t == KD * KD - 1),
                    )

            for pair in range(I // 2):
                xtile = xtiles[pair]
                psums = []
                for half in range(NHALF):
                    row = []
                    for par in range(2):
                        ps = psum_pool.tile([128, NTILE], mybir.dt.float32,
                                            name=f"ps_{pair}_{half}_{par}", tag="ps")
                        row.append(ps)
                    psums.append(row)

                if pair == 0:
                    if WARMUP:
                        # Zero-valued warm matmuls rotating over the 4 real
                        # chains (start=True on each chain's first, so the
                        # chain begins as explicit zeros; real taps then
                        # accumulate with start=False).  All warm matmuls use
                        # row group h0 so they SERIALIZE in the PE (two
                        # row-group-disjoint matmuls run concurrently, which
                        # would halve the busy window the warm-up exists to
                        # fill): N x 427ns of continuous PE activity from
                        # ~7.2us until the first input DMA lands ~10us.
                        for i in range(N_WARM):
                            h, par = divmod(i % 4, 2)
                            nc.tensor.matmul(
                                psums[h][par][:, :],
                                wtile[0:64, 0:128],
                                wtile[0:64, 128:640],
                                start=(i < 4), stop=False,
                            )
                    # half-major: half 0 only needs the first row-split load
                    sched = [(h, par, t) for h in range(NHALF)
                             for t in range(KD * KD) for par in range(2)]
                elif pair == I // 2 - 1:
                    # Last pair: each 512-col chain is split into two
                    # independent 256-col accumulation groups, processed as
                    # four staggered duos (par0+par1 concurrent).  Each duo's
                    # outputs copy+store while the next duo's matmuls run, so
                    # after the very last matmul only a 256-col copy + 65KB
                    # store remain (~2.5us tail instead of ~5us).
                    for h, cg in ((0, 0), (1, 0), (0, 1), (1, 1)):
                        c0 = cg * (NTILE // 2)
                        r0 = h * ROWS_PER_TILE + cg * (ROWS_PER_TILE // 2)
                        for t in range(KD * KD):
                            kh, kw = divmod(t, KD)
                            for par in range(2):
                                p0 = par * 64
                                nc.tensor.matmul(
                                    psums[h][par][:, c0:c0 + NTILE // 2],
                                    ktile[p0:p0 + 64, t, :],
                                    xtile[p0:p0 + 64,
                                          r0 + kh:r0 + kh + ROWS_PER_TILE // 2,
                                          kw:kw + W],
                                    start=(t == 0), stop=(t == KD * KD - 1),
                                )
                        for par in range(2):
                            i_img = pair * 2 + par
                            otile = opool.tile([128, NTILE // 2], out_dt,
                                               name=f"og_{h}_{cg}_{par}", tag="o")
                            nc.vector.tensor_copy(
                                out=otile[:, :],
                                in_=psums[h][par][:, c0:c0 + NTILE // 2])
                            eng = nc.sync if par == 0 else nc.scalar
                            eng.dma_start(
                                out=out[i_img, :,
                                        h * NTILE + c0:h * NTILE + c0 + NTILE // 2],
                                in_=otile[:, :],
                            )
                    continue
                else:
                    # Taps 0-6 tap-major over all 4 psum chains (maximizes
                    # independent work in the PE queue so LDWEIGHTS stays
                    # hidden); the final two taps chain-grouped so the 4
                    # chains *finish* staggered and their PSUM->SBUF copies
                    # overlap the remaining matmuls instead of piling up
                    # after the last one.
                    sched = [(h, par, t) for t in range(KD * KD - 2)
                             for h in range(NHALF) for par in range(2)]
                    sched += [(h, par, t) for h in range(NHALF)
                              for par in range(2)
                              for t in (KD * KD - 2, KD * KD - 1)]
                emit_mm(psums, xtile, sched, warmed=(pair == 0 and WARMUP))

                # Stores alternate between the Sync and Scalar HWDGE queues:
                # each DMA_DIRECT2D issue costs ~600ns on its engine, so
                # pairing them across two engines halves the issue
                # serialization (the Scalar engine has issued all its input
                # loads by ~10.4us, well before the first store at ~13us).
                for half in range(NHALF):
                    for par in range(2):
                        i_img = pair * 2 + par
                        st_idx = pair * 4 + half * 2 + par
                        st_eng = nc.sync if st_idx % 2 == 0 else nc.scalar
                        otile = opool.tile([128, NTILE], out_dt,
                                           name=f"o_{pair}_{half}_{par}", tag="o")
                        nc.vector.tensor_copy(out=otile[:, :],
                                              in_=psums[half][par][:, :])
                        st_eng.dma_start(
                            out=out[i_img, :, half * NTILE:(half + 1) * NTILE],
                            in_=otile[:, :],
                        )
    nc.compile()
    return nc


def _prep_core_inputs(x_b: np.ndarray, k_b: np.ndarray):
    """x_b (16,32,32,64) f32, k_b (3,3,64,128) f32 -> device layouts."""
    np_in = np.float16 if MM_DTYPE == "f16" else np.float32
    xpad = np.zeros((I, HP, WP, C), dtype=np_in)
    xpad[:, 1:H + 1, 1:W + 1, :] = x_b
    # (I, HP, WP, C) -> (I, C, HP, WP) -> (I//2, 2*C, HP*WP)
    xp = np.ascontiguousarray(xpad.transpose(0, 3, 1, 2)).reshape(I // 2, 2 * C, HP * WP)

    kc = k_b.reshape(KD * KD, C, F)                       # (9, 64, 128)
    kdup = np.concatenate([kc, kc], axis=1)               # (9, 128, 128)
    kd = np.ascontiguousarray(kdup.transpose(1, 0, 2)).reshape(128, KD * KD * F)
    if W_DTYPE == "f16" or MM_DTYPE == "f16":
        kd = kd.astype(np.float16)
    return {
        "xp": xp,
        "xp0a": np.ascontiguousarray(xp[0][:, :18 * WP]),
        "xp0b": np.ascontiguousarray(xp[0][:, 18 * WP:]),
        "kdf": kd,
        "kd0": np.ascontiguousarray(kd[:, :F]),
        "kdR": np.ascontiguousarray(kd[:, F:]),
    }


def kernel(**inputs) -> np.ndarray:
    global _CACHED_NC, LAST_RESULTS
    x = np.asarray(inputs["x"], dtype=np.float32)
    k = np.asarray(inputs["kernel"], dtype=np.float32)

    if _CACHED_NC is None:
        _CACHED_NC = _build_nc()
    nc = _CACHED_NC

    in_maps = [_prep_core_inputs(x[b], k[b]) for b in range(B)]
    res = run_bass_kernel_spmd(nc, in_maps, core_ids=list(range(N_CORES)))
    LAST_RESULTS = res

    outs = []
    for b in range(B):
        o = np.asarray(res.results[b]["out"], dtype=np.float32)  # (16, 128, 1024)
        o = o.transpose(0, 2, 1).reshape(I, H, W, F)       # (16, 32, 32, 128)
        outs.append(o)
    return np.ascontiguousarray(np.stack(outs, axis=0))

